# revision 9
# baseline (speedup 1.0000x reference)
"""AttnBlock (GroupNorm + spatial self-attention + proj + residual) on 8 TRN2 cores.

Problem shapes (hardcoded): x (4, 512, 64, 64) fp32, 1x1-conv weights (512, 512).

Sharding: 8 cores = (batch b in 0..3) x (query half qh in 0..1). Attention is
permutation-invariant over key positions, so each core receives its batch's
x rotated along the flattened spatial axis so that its own 2048 query
positions are always columns 0:2048 -- the compiled NEFF is identical on all
cores (pure SPMD, no collectives). Each core computes GroupNorm, q for its
half, k/v for all 4096 positions, attention for its 2048 queries, the proj
conv and the residual, producing a (512, 2048) fp32 shard.

Numerics: GroupNorm/softmax statistics in fp32; all big matmuls use fp16
operands with fp32 PSUM accumulation (PE multiplies at FP22 internally).
exp(scale*s - 4) runs on the scalar engine directly out of PSUM; the
constant offset cancels in the softmax ratio. The v-bias is folded into
bp' = Wp @ bv + bp on the host; the softmax denominator divides the
*projected* output (division by S commutes with the channel contraction).
"""

from contextlib import ExitStack

import numpy as np

import concourse.bacc as bacc
import concourse.bass as bass
import concourse.mybir as mybir
import concourse.tile as tile
from concourse.bass_utils import run_bass_kernel_spmd

F32 = mybir.dt.float32
F16 = mybir.dt.float16

C = 512          # channels
N = 4096         # spatial positions (64*64)
NQ = 2048        # query positions per core
P = 128          # partitions
CT = C // P      # 4 channel tiles
NB = 512         # matmul free-dim block
NJ = N // P      # 32 key tiles
G = 32           # groups
GS = C // G      # 16 channels per group
GPT = P // GS    # 8 groups per channel tile
EPS = 1e-6
SCALE = float(C) ** -0.5
EXP_BIAS = -4.0  # constant max-proxy; cancels in the softmax ratio

N_CORES = 8


def _emit(ctx: ExitStack, tc: tile.TileContext):
    nc = tc.nc
    x_d = nc.declare_dram_parameter("x", [C, N], F32, isOutput=False)
    wqT_d = nc.declare_dram_parameter("wqT", [C, C], F16, isOutput=False)
    wkT_d = nc.declare_dram_parameter("wkT", [C, C], F16, isOutput=False)
    wvT_d = nc.declare_dram_parameter("wvT", [C, C], F16, isOutput=False)
    wpT_d = nc.declare_dram_parameter("wpT", [C, C], F16, isOutput=False)
    bq_d = nc.declare_dram_parameter("bq", [C], F32, isOutput=False)
    bk_d = nc.declare_dram_parameter("bk", [C], F32, isOutput=False)
    bp2_d = nc.declare_dram_parameter("bp2", [C], F32, isOutput=False)
    gamma_d = nc.declare_dram_parameter("gamma", [C], F32, isOutput=False)
    beta_d = nc.declare_dram_parameter("beta", [C], F32, isOutput=False)
    mask_d = nc.declare_dram_parameter("gmask", [P, GPT], F32, isOutput=False)
    expand_d = nc.declare_dram_parameter("gexpand", [GPT, P], F32, isOutput=False)
    out_d = nc.declare_dram_parameter("out", [C, NQ], F32, isOutput=True)

    consts = ctx.enter_context(tc.tile_pool(name="consts", bufs=1))
    big = ctx.enter_context(tc.tile_pool(name="big", bufs=1))
    stage = ctx.enter_context(tc.tile_pool(name="stage", bufs=2))
    gn_small = ctx.enter_context(tc.tile_pool(name="gn_small", bufs=2))
    exp_pool = ctx.enter_context(tc.tile_pool(name="exp_pool", bufs=4))
    att_sb_pool = ctx.enter_context(tc.tile_pool(name="att_sb_pool", bufs=2))
    out_pool = ctx.enter_context(tc.tile_pool(name="out_pool", bufs=4))
    ps_mm = ctx.enter_context(tc.tile_pool(name="ps_mm", bufs=3, space="PSUM"))
    ps_att = ctx.enter_context(tc.tile_pool(name="ps_att", bufs=1, space="PSUM"))

    # ---- constants ----
    w_sb = {}
    for wname, w_ap in (("q", wqT_d), ("k", wkT_d), ("v", wvT_d), ("p", wpT_d)):
        for t in range(CT):
            tl = consts.tile([P, C], F16, name=f"w{wname}_{t}", tag=f"w{wname}_{t}")
            nc.sync.dma_start(out=tl, in_=w_ap[t * P:(t + 1) * P, :])
            w_sb[wname, t] = tl

    def load_vec(ap, nm):
        r = ap[:].rearrange("(t p) -> t p", p=P)
        tiles = []
        for t in range(CT):
            tl = consts.tile([P, 1], F32, name=f"{nm}_{t}", tag=f"{nm}_{t}")
            nc.sync.dma_start(out=tl, in_=r[t][:, None])
            tiles.append(tl)
        return tiles

    bq_sb = load_vec(bq_d, "bq")
    bk_sb = load_vec(bk_d, "bk")
    bp2_sb = load_vec(bp2_d, "bp2")
    gamma_sb = load_vec(gamma_d, "gamma")
    beta_sb = load_vec(beta_d, "beta")

    mask_sb = consts.tile([P, GPT], F32, name="mask_sb", tag="mask_sb")
    nc.sync.dma_start(out=mask_sb, in_=mask_d[:, :])
    expand_sb = consts.tile([GPT, P], F32, name="expand_sb", tag="expand_sb")
    nc.sync.dma_start(out=expand_sb, in_=expand_d[:, :])
    ones_j = consts.tile([P, P], F16, name="ones_j", tag="ones_j")
    nc.vector.memset(ones_j, 1.0)
    expbias_sb = consts.tile([P, 1], F32, name="expbias_sb", tag="expbias_sb")
    nc.vector.memset(expbias_sb, EXP_BIAS)

    # ---- persistent big tensors ----
    h_sb = [big.tile([P, N], F16, name=f"h_{t}", tag=f"h_{t}") for t in range(CT)]
    k_sb = [big.tile([P, N], F16, name=f"k_{t}", tag=f"k_{t}") for t in range(CT)]
    q_sb = [big.tile([P, NQ], F16, name=f"q_{t}", tag=f"q_{t}") for t in range(CT)]
    vt_sb = big.tile([P, NJ, C], F16, name="vt_sb", tag="vt_sb")

    # ---- phase 1: GroupNorm ----
    for t in range(CT):
        xs = stage.tile([P, N], F32, name=f"xs_{t}", tag="xs")
        nc.sync.dma_start(out=xs, in_=x_d[t * P:(t + 1) * P, :])
        # per-channel mean/var via bn_stats over 8 chunks of 512
        st = gn_small.tile([P, N // NB, 6], F32, name=f"st_{t}", tag="st")
        xs_c = xs.rearrange("p (c f) -> p c f", f=NB)
        for cchunk in range(N // NB):
            nc.vector.bn_stats(out=st[:, cchunk, :], in_=xs_c[:, cchunk, :])
        mv = gn_small.tile([P, 2], F32, name=f"mv_{t}", tag="mv")
        nc.vector.bn_aggr(out=mv, in_=st)
        # ms2 = [mean_c, E[x^2]_c]
        ms2 = gn_small.tile([P, 2], F32, name=f"ms2_{t}", tag="ms2")
        nc.vector.tensor_copy(out=ms2[:, 0:1], in_=mv[:, 0:1])
        nc.vector.tensor_tensor(ms2[:, 1:2], mv[:, 0:1], mv[:, 0:1],
                                mybir.AluOpType.mult)
        nc.vector.tensor_add(ms2[:, 1:2], ms2[:, 1:2], mv[:, 1:2])
        # group-average across the 16-channel partition runs: mask matmul (fp32)
        gps = ps_mm.tile([GPT, 2], F32, name=f"gps_{t}", tag="mm")
        nc.tensor.matmul(gps, lhsT=mask_sb, rhs=ms2, start=True, stop=True)
        gmv = gn_small.tile([GPT, 2], F32, name=f"gmv_{t}", tag="gmv")
        nc.vector.tensor_copy(out=gmv, in_=gps)
        # vpe = var_g + eps ; rstd via ACT sqrt + reciprocal + one Newton step
        vpe = gn_small.tile([GPT, 1], F32, name=f"vpe_{t}", tag="vpe")
        nc.vector.tensor_tensor(vpe, gmv[:, 0:1], gmv[:, 0:1], mybir.AluOpType.mult)
        nc.vector.tensor_tensor(vpe, gmv[:, 1:2], vpe, mybir.AluOpType.subtract)
        nc.vector.tensor_scalar_add(vpe, vpe, EPS)
        sd = gn_small.tile([GPT, 1], F32, name=f"sd_{t}", tag="sd")
        nc.scalar.sqrt(out=sd, in_=vpe)
        y0 = gn_small.tile([GPT, 1], F32, name=f"y0_{t}", tag="y0")
        nc.vector.reciprocal(out=y0, in_=sd)
        t1 = gn_small.tile([GPT, 1], F32, name=f"t1_{t}", tag="t1")
        nc.vector.tensor_tensor(t1, y0, y0, mybir.AluOpType.mult)
        nc.vector.tensor_tensor(t1, t1, vpe, mybir.AluOpType.mult)
        nc.vector.tensor_scalar(t1, t1, -0.5, 1.5,
                                mybir.AluOpType.mult, mybir.AluOpType.add)
        grs = gn_small.tile([GPT, 2], F32, name=f"grs_{t}", tag="grs")
        nc.vector.tensor_copy(out=grs[:, 0:1], in_=gmv[:, 0:1])
        nc.vector.tensor_tensor(grs[:, 1:2], y0, t1, mybir.AluOpType.mult)
        # expand group stats back to channels: (GPT,P).T @ (GPT,2) -> (P,2)
        cps = ps_mm.tile([P, 2], F32, name=f"cps_{t}", tag="mm")
        nc.tensor.matmul(cps, lhsT=expand_sb, rhs=grs, start=True, stop=True)
        cms = gn_small.tile([P, 2], F32, name=f"cms_{t}", tag="cms")
        nc.vector.tensor_copy(out=cms, in_=cps)
        a_t = gn_small.tile([P, 1], F32, name=f"a_{t}", tag="a")
        nc.vector.tensor_tensor(a_t, gamma_sb[t], cms[:, 1:2], mybir.AluOpType.mult)
        b_t = gn_small.tile([P, 1], F32, name=f"b_{t}", tag="b")
        nc.vector.tensor_tensor(b_t, cms[:, 0:1], a_t, mybir.AluOpType.mult)
        nc.vector.tensor_tensor(b_t, beta_sb[t], b_t, mybir.AluOpType.subtract)
        # h = x*A + B, cast to fp16
        nc.vector.tensor_scalar(h_sb[t], xs, a_t, b_t,
                                mybir.AluOpType.mult, mybir.AluOpType.add)

    # ---- phase 2: q, k, vT convs ----
    ident = mybir.ActivationFunctionType.Identity
    for co in range(CT):
        for nb in range(N // NB):
            ps = ps_mm.tile([P, NB], F32, name=f"kps_{co}_{nb}", tag="mm")
            for ci in range(CT):
                nc.tensor.matmul(ps, lhsT=w_sb["k", ci][:, co * P:(co + 1) * P],
                                 rhs=h_sb[ci][:, nb * NB:(nb + 1) * NB],
                                 start=(ci == 0), stop=(ci == CT - 1))
            nc.scalar.activation(out=k_sb[co][:, nb * NB:(nb + 1) * NB], in_=ps,
                                 func=ident, bias=bk_sb[co], scale=1.0)
    for co in range(CT):
        for nb in range(NQ // NB):
            ps = ps_mm.tile([P, NB], F32, name=f"qps_{co}_{nb}", tag="mm")
            for ci in range(CT):
                nc.tensor.matmul(ps, lhsT=w_sb["q", ci][:, co * P:(co + 1) * P],
                                 rhs=h_sb[ci][:, nb * NB:(nb + 1) * NB],
                                 start=(ci == 0), stop=(ci == CT - 1))
            nc.scalar.activation(out=q_sb[co][:, nb * NB:(nb + 1) * NB], in_=ps,
                                 func=ident, bias=bq_sb[co], scale=1.0)
    for j in range(NJ):
        ps = ps_mm.tile([P, C], F32, name=f"vps_{j}", tag="mm")
        for ci in range(CT):
            nc.tensor.matmul(ps, lhsT=h_sb[ci][:, j * P:(j + 1) * P],
                             rhs=w_sb["v", ci],
                             start=(ci == 0), stop=(ci == CT - 1))
        nc.scalar.copy(out=vt_sb[:, j, :], in_=ps)

    # ---- phase 3: attention + proj + epilogue, per query block ----
    # Software-pipelined emission: scores(j+1) is emitted before att(j) so the
    # PE never stalls on the ACT exp; the previous block's proj/epilogue tail
    # is emitted two j-steps into the next block.
    def emit_tail(ib, att_ps, s_ps):
        isl = slice(ib * NB, (ib + 1) * NB)
        rb = out_pool.tile([P, NB], F32, name=f"rb_{ib}", tag="rb", bufs=2)
        nc.vector.reciprocal(out=rb, in_=s_ps)
        att_sb = []
        for c in range(CT):
            asb = att_sb_pool.tile([P, NB], F16, name=f"attsb_{ib}_{c}",
                                   tag=f"asb{c}")
            nc.scalar.copy(out=asb, in_=att_ps[c])
            att_sb.append(asb)
        for co in range(CT):
            xres = out_pool.tile([P, NB], F32, name=f"xres_{ib}_{co}", tag="xres")
            nc.gpsimd.dma_start(out=xres, in_=x_d[co * P:(co + 1) * P, isl])
            pp = ps_mm.tile([P, NB], F32, name=f"pp_{ib}_{co}", tag="mm")
            for ci in range(CT):
                nc.tensor.matmul(pp, lhsT=w_sb["p", ci][:, co * P:(co + 1) * P],
                                 rhs=att_sb[ci],
                                 start=(ci == 0), stop=(ci == CT - 1))
            fin = out_pool.tile([P, NB], F32, name=f"fin_{ib}_{co}", tag="fin")
            nc.vector.tensor_tensor(fin, pp, rb, mybir.AluOpType.mult)
            nc.vector.tensor_scalar_add(fin, fin, bp2_sb[co])
            nc.vector.tensor_add(fin, fin, xres)
            nc.sync.dma_start(out=out_d[co * P:(co + 1) * P, isl], in_=fin)

    pending = None
    for ib in range(NQ // NB):
        isl = slice(ib * NB, (ib + 1) * NB)
        att_ps = [ps_att.tile([P, NB], F32, name=f"attps_{ib}_{c}", tag=f"att{c}")
                  for c in range(CT)]
        s_ps = ps_att.tile([P, NB], F32, name=f"sps_{ib}", tag="s")
        ex_tiles = {}
        for j in range(NJ + 1):
            if j < NJ:
                sc = ps_mm.tile([P, NB], F32, name=f"sc_{ib}_{j}", tag="mm")
                for ci in range(CT):
                    nc.tensor.matmul(sc, lhsT=k_sb[ci][:, j * P:(j + 1) * P],
                                     rhs=q_sb[ci][:, isl],
                                     start=(ci == 0), stop=(ci == CT - 1))
                ex = exp_pool.tile([P, NB], F16, name=f"ex_{ib}_{j}", tag="exp")
                nc.scalar.activation(out=ex, in_=sc,
                                     func=mybir.ActivationFunctionType.Exp,
                                     bias=expbias_sb, scale=SCALE)
                ex_tiles[j] = ex
            if j >= 1:
                jp = j - 1
                ex = ex_tiles.pop(jp)
                for c in range(CT):
                    nc.tensor.matmul(att_ps[c],
                                     lhsT=vt_sb[:, jp, c * P:(c + 1) * P],
                                     rhs=ex, start=(jp == 0), stop=(jp == NJ - 1))
                nc.tensor.matmul(s_ps, lhsT=ones_j, rhs=ex,
                                 start=(jp == 0), stop=(jp == NJ - 1))
            if pending is not None and j == 2:
                emit_tail(*pending)
                pending = None
        pending = (ib, att_ps, s_ps)
    emit_tail(*pending)


_CACHED = None


def _build():
    global _CACHED
    if _CACHED is None:
        nc = bacc.Bacc()
        with tile.TileContext(nc) as tc, ExitStack() as ctx:
            _emit(ctx, tc)
        nc.finalize()
        _CACHED = nc
    return _CACHED


def _host_inputs(x, norm_gamma, norm_beta, Wq, bq, Wk, bk, Wv, bv, Wp, bp):
    common = {
        "wqT": np.ascontiguousarray(np.asarray(Wq, np.float32).T).astype(np.float16),
        "wkT": np.ascontiguousarray(np.asarray(Wk, np.float32).T).astype(np.float16),
        "wvT": np.ascontiguousarray(np.asarray(Wv, np.float32).T).astype(np.float16),
        "wpT": np.ascontiguousarray(np.asarray(Wp, np.float32).T).astype(np.float16),
        "bq": np.asarray(bq, np.float32),
        "bk": np.asarray(bk, np.float32),
        "bp2": (np.asarray(Wp, np.float64) @ np.asarray(bv, np.float64)
                + np.asarray(bp, np.float64)).astype(np.float32),
        "gamma": np.asarray(norm_gamma, np.float32),
        "beta": np.asarray(norm_beta, np.float32),
        "gmask": ((np.arange(P)[:, None] // GS == np.arange(GPT)[None, :])
                  .astype(np.float32) / GS),
        "gexpand": (np.arange(GPT)[:, None] == np.arange(P)[None, :] // GS)
                   .astype(np.float32),
    }
    xf = np.asarray(x, np.float32).reshape(4, C, N)
    in_maps = []
    for core in range(N_CORES):
        bi, qh = core // 2, core % 2
        xc = np.ascontiguousarray(np.roll(xf[bi], -qh * NQ, axis=1))
        in_maps.append({"x": xc, **common})
    return in_maps


def kernel(x, norm_gamma, norm_beta, Wq, bq, Wk, bk, Wv, bv, Wp, bp):
    x = np.asarray(x, np.float32)
    b, c, hh, ww = x.shape
    assert (b, c, hh * ww) == (4, C, N)
    nc = _build()
    in_maps = _host_inputs(x, norm_gamma, norm_beta,
                           Wq, bq, Wk, bk, Wv, bv, Wp, bp)
    res = run_bass_kernel_spmd(nc, in_maps, core_ids=list(range(N_CORES)))
    y = np.empty((4, C, N), np.float32)
    for core in range(N_CORES):
        bi, qh = core // 2, core % 2
        y[bi][:, qh * NQ:(qh + 1) * NQ] = res.results[core]["out"]
    return y.reshape(b, c, hh, ww)


# revision 20
# speedup vs baseline: 10644.7623x; 10644.7623x over previous
"""AttnBlock (GroupNorm + spatial self-attention + proj + residual) on 8 TRN2 cores.

Problem shapes (hardcoded): x (4, 512, 64, 64) fp32, 1x1-conv weights (512, 512).

Sharding: 8 cores = (batch b in 0..3) x (query half qh in 0..1). Attention is
permutation-invariant over key positions, so each core receives its batch's
x rotated along the flattened spatial axis so that its own 2048 query
positions are always columns 0:2048 -- the compiled NEFF is identical on all
cores (pure SPMD, no collectives). Each core computes GroupNorm, q for its
half, k/v for all 4096 positions, attention for its 2048 queries, the proj
conv and the residual, producing a (512, 2048) fp32 shard.

Numerics: GroupNorm/softmax statistics in fp32; all big matmuls use fp16
operands with fp32 PSUM accumulation (PE multiplies at FP22 internally).
exp(scale*s - 4) runs on the scalar engine directly out of PSUM; the
constant offset cancels in the softmax ratio. The v-bias is folded into
bp' = Wp @ bv + bp on the host; the softmax denominator divides the
*projected* output (division by S commutes with the channel contraction).
"""

from contextlib import ExitStack

import numpy as np

import concourse.bacc as bacc
import concourse.bass as bass
import concourse.mybir as mybir
import concourse.tile as tile
from concourse.bass_utils import run_bass_kernel_spmd

F32 = mybir.dt.float32
F16 = mybir.dt.float16

C = 512          # channels
N = 4096         # spatial positions (64*64)
NQ = 2048        # query positions per core
P = 128          # partitions
CT = C // P      # 4 channel tiles
NB = 512         # matmul free-dim block
NJ = N // P      # 32 key tiles
G = 32           # groups
GS = C // G      # 16 channels per group
GPT = P // GS    # 8 groups per channel tile
EPS = 1e-6
SCALE = float(C) ** -0.5
EXP_BIAS = -4.0  # constant max-proxy; cancels in the softmax ratio

N_CORES = 8


def _emit(ctx: ExitStack, tc: tile.TileContext, merged: bool):
    nc = tc.nc
    x_d = nc.declare_dram_parameter("x", [C, N], F32, isOutput=False)
    if merged:
        wmT_d = nc.declare_dram_parameter("wmT", [C, C], F16, isOutput=False)
    else:
        wqT_d = nc.declare_dram_parameter("wqT", [C, C], F16, isOutput=False)
        wkT_d = nc.declare_dram_parameter("wkT", [C, C], F16, isOutput=False)
    wvT_d = nc.declare_dram_parameter("wvT", [C, C], F16, isOutput=False)
    wpT_d = nc.declare_dram_parameter("wpT", [C, C], F16, isOutput=False)
    if not merged:
        bq_d = nc.declare_dram_parameter("bq", [C], F32, isOutput=False)
        bk_d = nc.declare_dram_parameter("bk", [C], F32, isOutput=False)
    bp2_d = nc.declare_dram_parameter("bp2", [C], F32, isOutput=False)
    gamma_d = nc.declare_dram_parameter("gamma", [C], F32, isOutput=False)
    beta_d = nc.declare_dram_parameter("beta", [C], F32, isOutput=False)
    mask_d = nc.declare_dram_parameter("gmask", [P, GPT], F32, isOutput=False)
    expand_d = nc.declare_dram_parameter("gexpand", [GPT, P], F32, isOutput=False)
    out_d = nc.declare_dram_parameter("out", [C, NQ], F32, isOutput=True)

    consts = ctx.enter_context(tc.tile_pool(name="consts", bufs=1))
    big = ctx.enter_context(tc.tile_pool(name="big", bufs=1))
    stage = ctx.enter_context(tc.tile_pool(name="stage", bufs=2))
    gn_small = ctx.enter_context(tc.tile_pool(name="gn_small", bufs=2))
    exp_pool = ctx.enter_context(tc.tile_pool(name="exp_pool", bufs=4))
    att_sb_pool = ctx.enter_context(tc.tile_pool(name="att_sb_pool", bufs=2))
    out_pool = ctx.enter_context(tc.tile_pool(name="out_pool", bufs=4))
    ps_mm = ctx.enter_context(tc.tile_pool(name="ps_mm", bufs=3, space="PSUM"))
    ps_att = ctx.enter_context(tc.tile_pool(name="ps_att", bufs=1, space="PSUM"))

    ident_f = mybir.ActivationFunctionType.Identity

    # ---- start the x stream immediately on the HWDGE (sync) queue; all
    # constant loads go through SWDGE (gpsimd) so they don't delay x ----
    xs_tiles = []
    for t in range(CT):
        xs = stage.tile([P, N], F32, name=f"xs_{t}", tag="xs")
        # chunked so bn_stats can start before the whole tile lands
        for ch in range(4):
            nc.sync.dma_start(out=xs[:, ch * (N // 4):(ch + 1) * (N // 4)],
                              in_=x_d[t * P:(t + 1) * P,
                                      ch * (N // 4):(ch + 1) * (N // 4)])
        xs_tiles.append(xs)

    # small GN constants first -- the first GN matmul waits on mask/expand
    mask_sb = consts.tile([P, GPT], F32, name="mask_sb", tag="mask_sb")
    nc.gpsimd.dma_start(out=mask_sb, in_=mask_d[:, :])
    expand_sb = consts.tile([GPT, P], F32, name="expand_sb", tag="expand_sb")
    nc.gpsimd.dma_start(out=expand_sb, in_=expand_d[:, :])

    def load_vec(ap, nm):
        r = ap[:].rearrange("(t p) -> t p", p=P)
        tiles = []
        for t in range(CT):
            tl = consts.tile([P, 1], F32, name=f"{nm}_{t}", tag=f"{nm}_{t}")
            nc.gpsimd.dma_start(out=tl, in_=r[t][:, None])
            tiles.append(tl)
        return tiles

    gamma_sb = load_vec(gamma_d, "gamma")
    beta_sb = load_vec(beta_d, "beta")
    if not merged:
        bq_sb = load_vec(bq_d, "bq")
        bk_sb = load_vec(bk_d, "bk")
    bp2_sb = load_vec(bp2_d, "bp2")

    # weights last, in first-use order (k conv runs first)
    w_sb = {}
    if merged:
        w_order = (("k", wmT_d), ("v", wvT_d), ("p", wpT_d))
    else:
        w_order = (("k", wkT_d), ("v", wvT_d), ("q", wqT_d), ("p", wpT_d))
    for wname, w_ap in w_order:
        for t in range(CT):
            tl = consts.tile([P, C], F16, name=f"w{wname}_{t}", tag=f"w{wname}_{t}")
            nc.gpsimd.dma_start(out=tl, in_=w_ap[t * P:(t + 1) * P, :])
            w_sb[wname, t] = tl
    ones_j = consts.tile([P, P], F16, name="ones_j", tag="ones_j")
    nc.vector.memset(ones_j, 1.0)
    expbias_sb = consts.tile([P, 1], F32, name="expbias_sb", tag="expbias_sb")
    nc.vector.memset(expbias_sb, EXP_BIAS)

    # ---- persistent big tensors ----
    h_sb = [big.tile([P, N], F16, name=f"h_{t}", tag=f"h_{t}") for t in range(CT)]
    k_sb = [big.tile([P, N], F16, name=f"k_{t}", tag=f"k_{t}") for t in range(CT)]
    if not merged:
        q_sb = [big.tile([P, NQ], F16, name=f"q_{t}", tag=f"q_{t}")
                for t in range(CT)]
    vt_sb = big.tile([P, NJ, C], F16, name="vt_sb", tag="vt_sb")

    # ---- phase 1: GroupNorm ----
    for t in range(CT):
        xs = xs_tiles[t]
        # per-channel mean/var via bn_stats over 8 chunks of 512
        st = gn_small.tile([P, N // NB, 6], F32, name=f"st_{t}", tag="st")
        xs_c = xs.rearrange("p (c f) -> p c f", f=NB)
        for cchunk in range(N // NB):
            nc.vector.bn_stats(out=st[:, cchunk, :], in_=xs_c[:, cchunk, :])
        mv = gn_small.tile([P, 2], F32, name=f"mv_{t}", tag="mv")
        nc.vector.bn_aggr(out=mv, in_=st)
        # ms2 = [mean_c, E[x^2]_c]
        ms2 = gn_small.tile([P, 2], F32, name=f"ms2_{t}", tag="ms2")
        nc.vector.tensor_copy(out=ms2[:, 0:1], in_=mv[:, 0:1])
        nc.vector.tensor_tensor(ms2[:, 1:2], mv[:, 0:1], mv[:, 0:1],
                                mybir.AluOpType.mult)
        nc.vector.tensor_add(ms2[:, 1:2], ms2[:, 1:2], mv[:, 1:2])
        # group-average across the 16-channel partition runs: mask matmul (fp32)
        gps = ps_mm.tile([GPT, 2], F32, name=f"gps_{t}", tag="mm")
        nc.tensor.matmul(gps, lhsT=mask_sb, rhs=ms2, start=True, stop=True)
        gmv = gn_small.tile([GPT, 2], F32, name=f"gmv_{t}", tag="gmv")
        nc.vector.tensor_copy(out=gmv, in_=gps)
        # vpe = var_g + eps ; rstd via ACT sqrt + reciprocal + one Newton step
        vpe = gn_small.tile([GPT, 1], F32, name=f"vpe_{t}", tag="vpe")
        nc.vector.tensor_tensor(vpe, gmv[:, 0:1], gmv[:, 0:1], mybir.AluOpType.mult)
        nc.vector.tensor_tensor(vpe, gmv[:, 1:2], vpe, mybir.AluOpType.subtract)
        nc.vector.tensor_scalar_add(vpe, vpe, EPS)
        sd = gn_small.tile([GPT, 1], F32, name=f"sd_{t}", tag="sd")
        nc.scalar.sqrt(out=sd, in_=vpe)
        y0 = gn_small.tile([GPT, 1], F32, name=f"y0_{t}", tag="y0")
        nc.vector.reciprocal(out=y0, in_=sd)
        t1 = gn_small.tile([GPT, 1], F32, name=f"t1_{t}", tag="t1")
        nc.vector.tensor_tensor(t1, y0, y0, mybir.AluOpType.mult)
        nc.vector.tensor_tensor(t1, t1, vpe, mybir.AluOpType.mult)
        nc.vector.tensor_scalar(t1, t1, -0.5, 1.5,
                                mybir.AluOpType.mult, mybir.AluOpType.add)
        grs = gn_small.tile([GPT, 2], F32, name=f"grs_{t}", tag="grs")
        nc.vector.tensor_copy(out=grs[:, 0:1], in_=gmv[:, 0:1])
        nc.vector.tensor_tensor(grs[:, 1:2], y0, t1, mybir.AluOpType.mult)
        # expand group stats back to channels: (GPT,P).T @ (GPT,2) -> (P,2)
        cps = ps_mm.tile([P, 2], F32, name=f"cps_{t}", tag="mm")
        nc.tensor.matmul(cps, lhsT=expand_sb, rhs=grs, start=True, stop=True)
        cms = gn_small.tile([P, 2], F32, name=f"cms_{t}", tag="cms")
        nc.vector.tensor_copy(out=cms, in_=cps)
        a_t = gn_small.tile([P, 1], F32, name=f"a_{t}", tag="a")
        nc.vector.tensor_tensor(a_t, gamma_sb[t], cms[:, 1:2], mybir.AluOpType.mult)
        b_t = gn_small.tile([P, 1], F32, name=f"b_{t}", tag="b")
        nc.vector.tensor_tensor(b_t, cms[:, 0:1], a_t, mybir.AluOpType.mult)
        nc.vector.tensor_tensor(b_t, beta_sb[t], b_t, mybir.AluOpType.subtract)
        # h = x*A + B, cast to fp16 (on ACT: DVE is busy with bn_stats)
        nc.scalar.activation(out=h_sb[t], in_=xs, func=ident_f,
                             bias=b_t, scale=a_t)

    # ---- phase 2: q, k, vT convs ----
    # Conv PSUM groups rotate over all 7 available banks (ps_mm's 3 plus the
    # 4 attention-accumulator banks, which are idle during this phase) so the
    # PE can run partial ci-accumulations for many outputs while late h tiles
    # are still being produced.
    conv_n = 0

    def conv_psum(nm, free):
        nonlocal conv_n
        conv_n += 1
        if conv_n % 7 < 3:
            return ps_mm.tile([P, free], F32, name=nm, tag="mm")
        return ps_att.tile([P, free], F32, name=nm, tag=f"att{conv_n % 7 - 3}")

    ident = mybir.ActivationFunctionType.Identity
    for co in range(CT):
        for nb in range(N // NB):
            ps = conv_psum(f"kps_{co}_{nb}", NB)
            for ci in range(CT):
                nc.tensor.matmul(ps, lhsT=w_sb["k", ci][:, co * P:(co + 1) * P],
                                 rhs=h_sb[ci][:, nb * NB:(nb + 1) * NB],
                                 start=(ci == 0), stop=(ci == CT - 1))
            if merged:
                nc.scalar.copy(out=k_sb[co][:, nb * NB:(nb + 1) * NB], in_=ps)
            else:
                nc.scalar.activation(out=k_sb[co][:, nb * NB:(nb + 1) * NB],
                                     in_=ps, func=ident, bias=bk_sb[co], scale=1.0)
    if not merged:
        for co in range(CT):
            for nb in range(NQ // NB):
                ps = conv_psum(f"qps_{co}_{nb}", NB)
                for ci in range(CT):
                    nc.tensor.matmul(ps,
                                     lhsT=w_sb["q", ci][:, co * P:(co + 1) * P],
                                     rhs=h_sb[ci][:, nb * NB:(nb + 1) * NB],
                                     start=(ci == 0), stop=(ci == CT - 1))
                nc.scalar.activation(out=q_sb[co][:, nb * NB:(nb + 1) * NB],
                                     in_=ps, func=ident, bias=bq_sb[co],
                                     scale=1.0)
    for j in range(NJ):
        ps = conv_psum(f"vps_{j}", C)
        for ci in range(CT):
            nc.tensor.matmul(ps, lhsT=h_sb[ci][:, j * P:(j + 1) * P],
                             rhs=w_sb["v", ci],
                             start=(ci == 0), stop=(ci == CT - 1))
        nc.scalar.copy(out=vt_sb[:, j, :], in_=ps)

    # ---- phase 3: attention + proj + epilogue, per query block ----
    # Software-pipelined emission: scores(j+1) is emitted before att(j) so the
    # PE never stalls on the ACT exp; the previous block's proj/epilogue tail
    # is emitted two j-steps into the next block.
    def emit_tail(ib, att_ps, s_ps):
        isl = slice(ib * NB, (ib + 1) * NB)
        rb = out_pool.tile([P, NB], F32, name=f"rb_{ib}", tag="rb", bufs=2)
        nc.vector.reciprocal(out=rb, in_=s_ps)
        att_sb = []
        for c in range(CT):
            asb = att_sb_pool.tile([P, NB], F16, name=f"attsb_{ib}_{c}",
                                   tag=f"asb{c}")
            nc.scalar.copy(out=asb, in_=att_ps[c])
            att_sb.append(asb)
        for co in range(CT):
            xres = out_pool.tile([P, NB], F32, name=f"xres_{ib}_{co}", tag="xres")
            nc.gpsimd.dma_start(out=xres, in_=x_d[co * P:(co + 1) * P, isl])
            pp = ps_mm.tile([P, NB], F32, name=f"pp_{ib}_{co}", tag="mm")
            for ci in range(CT):
                nc.tensor.matmul(pp, lhsT=w_sb["p", ci][:, co * P:(co + 1) * P],
                                 rhs=att_sb[ci],
                                 start=(ci == 0), stop=(ci == CT - 1))
            fin = out_pool.tile([P, NB], F32, name=f"fin_{ib}_{co}", tag="fin")
            nc.vector.tensor_tensor(fin, pp, rb, mybir.AluOpType.mult)
            nc.vector.tensor_scalar_add(fin, fin, bp2_sb[co])
            nc.vector.tensor_add(fin, fin, xres)
            nc.sync.dma_start(out=out_d[co * P:(co + 1) * P, isl], in_=fin)

    pending = None
    for ib in range(NQ // NB):
        isl = slice(ib * NB, (ib + 1) * NB)
        att_ps = [ps_att.tile([P, NB], F32, name=f"attps_{ib}_{c}", tag=f"att{c}")
                  for c in range(CT)]
        s_ps = ps_att.tile([P, NB], F32, name=f"sps_{ib}", tag="s")
        ex_tiles = {}
        for j in range(NJ + 1):
            if j < NJ:
                sc = ps_mm.tile([P, NB], F32, name=f"sc_{ib}_{j}", tag="mm")
                for ci in range(CT):
                    qrhs = h_sb[ci][:, isl] if merged else q_sb[ci][:, isl]
                    nc.tensor.matmul(sc, lhsT=k_sb[ci][:, j * P:(j + 1) * P],
                                     rhs=qrhs,
                                     start=(ci == 0), stop=(ci == CT - 1))
                ex = exp_pool.tile([P, NB], F16, name=f"ex_{ib}_{j}", tag="exp")
                nc.scalar.activation(out=ex, in_=sc,
                                     func=mybir.ActivationFunctionType.Exp,
                                     bias=expbias_sb, scale=SCALE)
                ex_tiles[j] = ex
            if j >= 1:
                jp = j - 1
                ex = ex_tiles.pop(jp)
                for c in range(CT):
                    nc.tensor.matmul(att_ps[c],
                                     lhsT=vt_sb[:, jp, c * P:(c + 1) * P],
                                     rhs=ex, start=(jp == 0), stop=(jp == NJ - 1))
                nc.tensor.matmul(s_ps, lhsT=ones_j, rhs=ex,
                                 start=(jp == 0), stop=(jp == NJ - 1))
            if pending is not None and j == 2:
                emit_tail(*pending)
                pending = None
        pending = (ib, att_ps, s_ps)
    emit_tail(*pending)


_CACHED = {}


def _build(merged=True):
    if merged not in _CACHED:
        nc = bacc.Bacc()
        with tile.TileContext(nc) as tc, ExitStack() as ctx:
            _emit(ctx, tc, merged)
        nc.finalize()
        _CACHED[merged] = nc
    return _CACHED[merged]


def _host_inputs(x, norm_gamma, norm_beta, Wq, bq, Wk, bk, Wv, bv, Wp, bp,
                 merged=None):
    if merged is None:
        merged = (not np.any(np.asarray(bq))) and (not np.any(np.asarray(bk)))
    common = {
        "wvT": np.ascontiguousarray(np.asarray(Wv, np.float32).T).astype(np.float16),
        "wpT": np.ascontiguousarray(np.asarray(Wp, np.float32).T).astype(np.float16),
        "bp2": (np.asarray(Wp, np.float64) @ np.asarray(bv, np.float64)
                + np.asarray(bp, np.float64)).astype(np.float32),
        "gamma": np.asarray(norm_gamma, np.float32),
        "beta": np.asarray(norm_beta, np.float32),
        "gmask": ((np.arange(P)[:, None] // GS == np.arange(GPT)[None, :])
                  .astype(np.float32) / GS),
        "gexpand": (np.arange(GPT)[:, None] == np.arange(P)[None, :] // GS)
                   .astype(np.float32),
    }
    if merged:
        common["wmT"] = (np.asarray(Wk, np.float64).T
                         @ np.asarray(Wq, np.float64)).astype(np.float16)
    else:
        common["wqT"] = np.ascontiguousarray(
            np.asarray(Wq, np.float32).T).astype(np.float16)
        common["wkT"] = np.ascontiguousarray(
            np.asarray(Wk, np.float32).T).astype(np.float16)
        common["bq"] = np.asarray(bq, np.float32)
        common["bk"] = np.asarray(bk, np.float32)
    xf = np.asarray(x, np.float32).reshape(4, C, N)
    in_maps = []
    for core in range(N_CORES):
        bi, qh = core // 2, core % 2
        xc = np.ascontiguousarray(np.roll(xf[bi], -qh * NQ, axis=1))
        in_maps.append({"x": xc, **common})
    return in_maps


def kernel(x, norm_gamma, norm_beta, Wq, bq, Wk, bk, Wv, bv, Wp, bp):
    x = np.asarray(x, np.float32)
    b, c, hh, ww = x.shape
    assert (b, c, hh * ww) == (4, C, N)
    merged = (not np.any(np.asarray(bq))) and (not np.any(np.asarray(bk)))
    nc = _build(merged)
    in_maps = _host_inputs(x, norm_gamma, norm_beta,
                           Wq, bq, Wk, bk, Wv, bv, Wp, bp, merged=merged)
    res = run_bass_kernel_spmd(nc, in_maps, core_ids=list(range(N_CORES)))
    y = np.empty((4, C, N), np.float32)
    for core in range(N_CORES):
        bi, qh = core // 2, core % 2
        y[bi][:, qh * NQ:(qh + 1) * NQ] = res.results[core]["out"]
    return y.reshape(b, c, hh, ww)
